# revision 1
# baseline (speedup 1.0000x reference)
"""Trainium2 Bass kernel for nn_Mlp_moe: dense patch-token MLP + top-1 gated
atom (expert) routing for 6 CLS task tokens.

Sharding over 8 NeuronCores:
  - Patch MLP: data-parallel over batch B=64 -> 8 batches (1568 patch tokens)
    per core. MLP weights replicated (SBUF-resident, bf16).
  - Atom/CLS part: hidden dim H=3072 sharded 8-way (384 per core); every core
    processes all 384 CLS tokens for all 5 atoms on its H-shard and emits a
    partial output summed on the host. Routing (gate logits/sigmoid/top-1
    masks) is computed on the host (it is O(B*6*D), negligible) and shipped
    as {0,1}/weight masks folded into the device compute.

Device compute is bf16 (PSUM accumulation is fp32; erf-Gelu on ScalarE is
~exact); outputs are fp32. Tokens always ride the free (moving) dim, so the
1568 per-core tokens need no 128-padding; 4 chunks of 392 columns.

DMA: per-engine queues serialize at ~300 GB/s with ~0.5us per-DMA overhead,
so inputs are packed host-side into few large partition-major transfers and
spread across the sync/vector/scalar/gpsimd queues in compute-need order.
"""

import numpy as np
import ml_dtypes

import concourse.bass as bass
import concourse.bacc as bacc
import concourse.mybir as mybir
from concourse import tile
from concourse.bass_utils import run_bass_kernel_spmd

NCORES = 8
B, NCLS, P, D, H = 64, 6, 196, 768, 3072
NA = 5
HSH = H // NCORES            # 384: per-core atom hidden shard
BPC = B // NCORES            # 8 batches per core
TPC = BPC * P                # 1568 patch tokens per core
NT = B * NCLS                # 384 cls tokens
DT = D // 128                # 6 d-tiles
HT = H // 128                # 24 h-tiles
HLT = NA * HSH // 128        # 15 atom h-shard tiles (a-major, 3 per atom)
KPA = HSH // 128             # 3 h-shard tiles per atom
CW = 392
NCH = 4
CHUNKS = [(i * CW, CW) for i in range(NCH)]

LEFT_KEYS = np.array([3, 4, 8, 9, 13, 14], dtype=np.int64)
RIGHT_KEYS = np.array([15, 20, 16, 21, 17, 22], dtype=np.int64)

BF16 = mybir.dt.bfloat16
F32 = mybir.dt.float32
AF = mybir.ActivationFunctionType

_CACHE = {}
LAST_RESULTS = None  # BassKernelResults of the most recent run (for profiling)


def _build_program():
    nc = bacc.Bacc(None, target_bir_lowering=False, debug=False,
                   num_devices=NCORES)

    # partition-major packed inputs (see host layouts in kernel())
    xT_d = nc.dram_tensor("xT", [128, NCH * DT * CW], BF16,
                          kind="ExternalInput")
    w1T_d = nc.dram_tensor("w1T", [128, 2, DT * 12 * 128], BF16,
                           kind="ExternalInput")
    b1T_d = nc.dram_tensor("b1T", [128, HT], F32, kind="ExternalInput")
    w2T_d = nc.dram_tensor("w2T", [128, HT * D], BF16, kind="ExternalInput")
    clsT_d = nc.dram_tensor("clsT", [128, DT * NT], BF16,
                            kind="ExternalInput")
    ainT_d = nc.dram_tensor("ainT", [DT, 128, NA * HSH], BF16,
                            kind="ExternalInput")
    ainbT_d = nc.dram_tensor("ainbT", [128, HLT], F32, kind="ExternalInput")
    aoutT_d = nc.dram_tensor("aoutT", [NA, 128, KPA * D], BF16,
                             kind="ExternalInput")
    msrc_d = nc.dram_tensor("msrc", [128, NA * NT], BF16,
                            kind="ExternalInput")
    mdw_d = nc.dram_tensor("mdw", [128, NA * NT], F32, kind="ExternalInput")
    poutT_d = nc.dram_tensor("poutT", [DT, 128, TPC], F32,
                             kind="ExternalOutput")
    cpartT_d = nc.dram_tensor("cpartT", [DT, 128, NT], F32,
                              kind="ExternalOutput")

    with tile.TileContext(nc) as tc:
        with (
            tc.tile_pool(name="w", bufs=1) as wp,
            tc.tile_pool(name="gat", bufs=1) as gp,
            tc.tile_pool(name="sel", bufs=1) as sp,
            tc.tile_pool(name="hida", bufs=1) as hp,
            tc.tile_pool(name="xin", bufs=2) as xp,
            tc.tile_pool(name="g1", bufs=24) as g1p,
            tc.tile_pool(name="ostg", bufs=2) as op,
            tc.tile_pool(name="ps", bufs=8, space="PSUM") as pp,
        ):
            # ---- resident loads, 3 parallel queues, need-ordered ----
            # sync queue: cls + all atom tensors + masks
            clsT = wp.tile([128, DT * NT], BF16, tag="cls", name="cls")
            nc.sync.dma_start(clsT[:], clsT_d[:])
            ainbT = wp.tile([128, HLT], F32, tag="ainb", name="ainb")
            nc.sync.dma_start(ainbT[:], ainbT_d[:])
            ainT = [wp.tile([128, NA * HSH], BF16, tag=f"ain{d}",
                            name=f"ain{d}") for d in range(DT)]
            for d in range(4):
                nc.sync.dma_start(ainT[d][:], ainT_d[d])
            msrc = wp.tile([128, NA * NT], BF16, tag="ms", name="ms")
            nc.sync.dma_start(msrc[:], msrc_d[:])
            mdw = wp.tile([128, NA * NT], F32, tag="md", name="md")
            nc.sync.dma_start(mdw[:], mdw_d[:])
            aoutT = [wp.tile([128, KPA * D], BF16, tag=f"ao{a}",
                             name=f"ao{a}") for a in range(NA)]
            for a in range(NA):
                nc.sync.dma_start(aoutT[a][:], aoutT_d[a])

            # scalar queue: patch weights up front (issue cost only ~0.7us
            # each; transfers run async on the DGE engines), then the gelus
            w1T = [wp.tile([128, DT * 12 * 128], BF16, tag=f"w1{q}",
                           name=f"w1{q}") for q in range(2)]
            nc.scalar.dma_start(w1T[0][:], w1T_d[:, 0])
            b1T = wp.tile([128, HT], F32, tag="b1", name="b1")
            nc.scalar.dma_start(b1T[:], b1T_d[:])
            nc.scalar.dma_start(w1T[1][:], w1T_d[:, 1])
            w2T = wp.tile([128, HT * D], BF16, tag="w2", name="w2")

            # gpsimd queue: ain tail first, then x chunks (+ outputs later)
            nc.gpsimd.dma_start(ainT[4][:], ainT_d[4])
            nc.gpsimd.dma_start(ainT[5][:], ainT_d[5])

            def load_x(ci):
                xa = xp.tile([128, DT * CW], BF16, tag="x", name="x")
                nc.gpsimd.dma_start(
                    xa[:], xT_d[:, ci * DT * CW:(ci + 1) * DT * CW])
                return xa

            xs_pre = [load_x(0), load_x(1)]

            # ---- phase A: atom in-GEMM + gelu, a-major ----
            G = {}
            for a in range(NA):
                for k in range(KPA):
                    ps = pp.tile([128, 512], F32, tag="ps", name="ps")
                    c0 = a * HSH + k * 128
                    for d in range(DT):
                        nc.tensor.matmul(ps[:, :NT],
                                         ainT[d][:, c0:c0 + 128],
                                         clsT[:, d * NT:(d + 1) * NT],
                                         start=(d == 0), stop=(d == DT - 1))
                    hl = a * KPA + k
                    g = gp.tile([128, NT], BF16, tag=f"g{a}_{k}",
                                name=f"g{a}_{k}")
                    nc.scalar.activation(g[:], ps[:, :NT], AF.Gelu,
                                         bias=ainbT[:, hl:hl + 1])
                    G[(a, k)] = g

            # w2 load issues here so the 15 phase-A gelus above are not
            # queued behind a semaphore-blocked dma_start on ScalarE
            nc.scalar.dma_start(w2T[:], w2T_d[:])

            # ---- phase B: src-select + dst-mask (DVE, overlaps patch) ----
            for k in range(KPA):
                sel = sp.tile([128, NT], BF16, tag=f"sel{k}", name=f"sel{k}")
                tmp = sp.tile([128, NT], BF16, tag="tmp", name="tmp")
                nc.vector.tensor_mul(sel[:], G[(0, k)][:], msrc[:, :NT])
                for a in range(1, NA):
                    nc.vector.tensor_mul(tmp[:], G[(a, k)][:],
                                         msrc[:, a * NT:(a + 1) * NT])
                    nc.vector.tensor_add(sel[:], sel[:], tmp[:])
                for a in range(NA):
                    h = hp.tile([128, NT], BF16, tag=f"hida{a}_{k}",
                                name=f"hida{a}_{k}")
                    nc.vector.tensor_mul(h[:], sel[:],
                                         mdw[:, a * NT:(a + 1) * NT])
                    G[("hida", a, k)] = h

            def patch_chunk(ci, xa):
                c0, cw = CHUNKS[ci]
                g1s = []
                for h in range(HT):
                    ps = pp.tile([128, 512], F32, tag="ps", name="ps")
                    q, hh = divmod(h, 12)
                    for d in range(DT):
                        nc.tensor.matmul(
                            ps[:, :cw],
                            w1T[q][:, d * 1536 + hh * 128:
                                   d * 1536 + (hh + 1) * 128],
                            xa[:, d * CW:d * CW + cw],
                            start=(d == 0), stop=(d == DT - 1))
                    g1 = g1p.tile([128, CW], BF16, tag="g1", name="g1")
                    nc.scalar.activation(g1[:, :cw], ps[:, :cw], AF.Gelu,
                                         bias=b1T[:, h:h + 1])
                    g1s.append(g1)
                for dp in range(DT):
                    ps = pp.tile([128, 512], F32, tag="ps", name="ps")
                    for h in range(HT):
                        nc.tensor.matmul(
                            ps[:, :cw],
                            w2T[:, h * D + dp * 128:h * D + (dp + 1) * 128],
                            g1s[h][:, :cw],
                            start=(h == 0), stop=(h == HT - 1))
                    stg = op.tile([128, CW], F32, tag="ostg", name="ostg")
                    nc.vector.tensor_copy(stg[:, :cw], ps[:, :cw])
                    nc.gpsimd.dma_start(poutT_d[dp][:, c0:c0 + cw],
                                        stg[:, :cw])

            # ---- patch chunk 0, then atom out-GEMM, then chunks 1..3 ----
            patch_chunk(0, xs_pre[0])

            for dp in range(DT):
                ps = pp.tile([128, 512], F32, tag="ps", name="ps")
                n = 0
                for a in range(NA):
                    for k in range(KPA):
                        nc.tensor.matmul(
                            ps[:, :NT],
                            aoutT[a][:, k * D + dp * 128:
                                     k * D + (dp + 1) * 128],
                            G[("hida", a, k)][:],
                            start=(n == 0), stop=(n == NA * KPA - 1))
                        n += 1
                stg = op.tile([128, CW], F32, tag="ostg", name="ostg")
                nc.vector.tensor_copy(stg[:, :NT], ps[:, :NT])
                nc.gpsimd.dma_start(cpartT_d[dp], stg[:, :NT])

            xs_pre.append(load_x(2))
            patch_chunk(1, xs_pre[1])
            xs_pre.append(load_x(3))
            patch_chunk(2, xs_pre[2])
            patch_chunk(3, xs_pre[3])

    nc.compile()
    return nc


def _sigmoid(x):
    out = np.empty_like(x)
    pos = x >= 0
    out[pos] = 1.0 / (1.0 + np.exp(-x[pos]))
    ex = np.exp(x[~pos])
    out[~pos] = ex / (1.0 + ex)
    return out


def kernel(x, patch_w1, patch_b1, patch_w2, patch_b2, gate_delta,
           atom_in_w, atom_in_b, atom_out_w, atom_out_b):
    x = np.asarray(x, dtype=np.float32)
    patch_w1 = np.asarray(patch_w1, dtype=np.float32)
    patch_b1 = np.asarray(patch_b1, dtype=np.float32)
    patch_w2 = np.asarray(patch_w2, dtype=np.float32)
    patch_b2 = np.asarray(patch_b2, dtype=np.float32)
    gate_delta = np.asarray(gate_delta, dtype=np.float32)
    atom_in_w = np.asarray(atom_in_w, dtype=np.float32)
    atom_in_b = np.asarray(atom_in_b, dtype=np.float32)
    atom_out_w = np.asarray(atom_out_w, dtype=np.float32)
    atom_out_b = np.asarray(atom_out_b, dtype=np.float32)

    bf = ml_dtypes.bfloat16

    # ---- host routing (tiny) ----
    cls3 = x[:, :NCLS, :]                                   # [B, 6, D]
    logits = np.einsum("bnd,nd->bn", cls3, gate_delta)      # [B, 6] f32
    choose_left = logits >= 0
    p_left = _sigmoid(logits)
    wgt = np.where(choose_left, p_left, 1.0 - p_left).astype(np.float32)
    keys = np.where(choose_left, LEFT_KEYS[None, :], RIGHT_KEYS[None, :])
    src = (keys // NA).reshape(-1)                          # [384]
    dst = (keys % NA).reshape(-1)
    wflat = wgt.reshape(-1)                                 # [384]

    msrc = (src[None, :] == np.arange(NA)[:, None]).astype(np.float32)
    mdw = (dst[None, :] == np.arange(NA)[:, None]) * wflat[None, :]
    msrc_rep = np.ascontiguousarray(
        np.broadcast_to(msrc.reshape(1, NA * NT), (128, NA * NT))).astype(bf)
    mdw_rep = np.ascontiguousarray(
        np.broadcast_to(mdw.reshape(1, NA * NT),
                        (128, NA * NT))).astype(np.float32)

    # ---- replicated tensors (partition-major packed) ----
    # clsT[p, d*NT + t] = cls[t, d*128+p]
    clsT = np.ascontiguousarray(
        cls3.reshape(NT, DT, 128).transpose(2, 1, 0)
    ).reshape(128, DT * NT).astype(bf)
    # w1T[p, q, d*1536 + hh*128 + m] = patch_w1[(q*12+hh)*128+m, d*128+p]
    w1T = np.ascontiguousarray(
        patch_w1.reshape(2, 12, 128, DT, 128).transpose(4, 0, 3, 1, 2)
    ).reshape(128, 2, DT * 12 * 128).astype(bf)
    b1T = np.ascontiguousarray(patch_b1.reshape(HT, 128).T)
    # w2T[p, h*D + dp*128 + m] = patch_w2[dp*128+m, h*128+p]
    w2T = np.ascontiguousarray(
        patch_w2.reshape(DT, 128, HT, 128).transpose(3, 2, 0, 1)
    ).reshape(128, HT * D).astype(bf)

    # ---- per-core tensors ----
    patch = x[:, NCLS:, :].reshape(NCORES, TPC, D)
    # xT[p, ci*DT*CW + d*CW + t] = patch[c][ci*CW+t, d*128+p]
    xT_all = np.ascontiguousarray(
        patch.reshape(NCORES, NCH, CW, DT, 128).transpose(0, 4, 1, 3, 2)
    ).reshape(NCORES, 128, NCH * DT * CW).astype(bf)

    ainT_all, ainbT_all, aoutT_all = [], [], []
    for c in range(NCORES):
        hsl = slice(HSH * c, HSH * (c + 1))
        # ainT[d, p, a*HSH + k*128 + m] = atom_in_w[a, hsl0 + k*128+m, d*128+p]
        ainT = np.ascontiguousarray(
            atom_in_w[:, hsl, :].reshape(NA, KPA, 128, DT, 128)
            .transpose(3, 4, 0, 1, 2)).reshape(DT, 128, NA * HSH).astype(bf)
        ainT_all.append(ainT)
        ainbT_all.append(np.ascontiguousarray(
            atom_in_b[:, hsl].reshape(HLT, 128).T))
        # aoutT[a, p, k*D + dp*128 + m] = atom_out_w[a, dp*128+m, hsl0+k*128+p]
        aoutT = np.ascontiguousarray(
            atom_out_w[:, :, hsl].reshape(NA, DT, 128, KPA, 128)
            .transpose(0, 4, 3, 1, 2)).reshape(NA, 128, KPA * D).astype(bf)
        aoutT_all.append(aoutT)

    in_maps = []
    for c in range(NCORES):
        in_maps.append({
            "xT": xT_all[c], "w1T": w1T, "b1T": b1T, "w2T": w2T,
            "clsT": clsT, "ainT": ainT_all[c], "ainbT": ainbT_all[c],
            "aoutT": aoutT_all[c], "msrc": msrc_rep, "mdw": mdw_rep,
        })

    nc = _CACHE.get("nc")
    if nc is None:
        nc = _build_program()
        _CACHE["nc"] = nc

    res = run_bass_kernel_spmd(nc, in_maps, core_ids=list(range(NCORES)))
    global LAST_RESULTS
    LAST_RESULTS = res

    # ---- host gather ----
    patch_out = np.empty((B, P, D), dtype=np.float32)
    for c in range(NCORES):
        poutT = res.results[c]["poutT"].reshape(D, TPC)
        patch_out[BPC * c:BPC * (c + 1)] = (
            poutT.T + patch_b2[None, :]).reshape(BPC, P, D)

    cpart = np.zeros((D, NT), dtype=np.float32)
    for c in range(NCORES):
        cpart += res.results[c]["cpartT"].reshape(D, NT)
    cls_out = cpart.T + wflat[:, None] * atom_out_b[dst, :]
    cls_out = cls_out.reshape(B, NCLS, D)

    return np.concatenate([cls_out, patch_out], axis=1)

